# revision 9
# baseline (speedup 1.0000x reference)
"""Bilateral filter (7x7, dilation 1) Trainium2 Bass kernel — v9.

Problem: input [2, 18, 1024, 1024] f32.
  filterable = input[:, :8]; params = -(input[:, 8:]**2)
  range coeffs = params[:, :8], sx = params[:, 8], sy = params[:, 9]
  out[c] = sum_taps w * f_c(shifted) / sum_taps w, c < 3
  w = exp(sum_c r_c (fn_c - f_c)^2 + sx dx^2 + sy dy^2), OOB taps masked.

Sharding: data-parallel over (batch, H): 8 cores, each 256 rows of one batch
image (+3 halo rows / cols, sentinel-padded host-side; 6-col left pad).

v9 = v8 + d2 pair sharing. d2 for taps o and -o is the same plane shifted
by -o: compute the (wide, 518-col) sub+Square once per +-pair, then the
partner tap reads a partition-shifted SBUF->SBUF DMA copy (row shift) with
a column-shifted view (col shift). The 3 boundary rows missing from the
row shift come from host-precomputed d2 strips ("ts" input — same spirit
as the host halo padding). Halves the DVE sub work and the ACT Square
work: 24 sub+Square per macro instead of 48.

Per tap: DVE rd+ = R+ * d2(view) -> PE: 8 neg-identity matmuls + Asp+
matmul accumulate -logw in a PSUM bank -> ACT Exp reads PSUM -> DVE
t3 = w * [f0,f1,f2] -> PE identity matmuls accumulate w|t3 into 4
persistent PSUM acc banks across all 49 taps (center = init matmuls).
R+ = p^2 via ACT Square; Asp+ built from 4x tensor_scalar + 2x adds;
sign lives in the -I stationary. F tiles: one f32->f16 conversion per
134-row slab, 7 row-shifted tiles are partition-offset SBUF DMA copies.
Sentinel 24.0 keeps p^2*d^2 finite in fp16 (0 * -inf = NaN in the PE).
"""

import sys

if "/opt/trn_rl_repo" not in sys.path:
    sys.path.insert(0, "/opt/trn_rl_repo")

import numpy as np

import concourse.bass as bass
import concourse.mybir as mybir
from concourse.bacc import Bacc
from concourse.masks import make_identity
from concourse.tile import TileContext

FP32 = mybir.dt.float32
F16 = mybir.dt.float16

B, C_ALL, H, W = 2, 18, 1024, 1024
CF = 8                      # filterable channels
CO = 3                      # output channels
KS, RAD = 7, 3
HC = H * B // 8             # 256 output rows per core
HIN = HC + 2 * RAD          # 262 input rows per core (halo padded host-side)
LPAD = 6                    # left col pad (covers wide-d2 -3 col reach)
WIN = LPAD + W + 6          # 1036 input cols per core
WC = 512                    # W chunk (= one PSUM bank of fp32)
NW = W // WC                # 2
NHB = HC // 128             # 2
WT = LPAD + WC + 6          # 524 = chunk + wide col halo (even)
WD = WC + 6                 # 518 = wide d2 width
SENT = 24.0                 # sentinel: exp(-p2*d^2 sum) == 0, finite in fp16
IDX4 = [3, 2, 1, 0, 1, 2, 3]                          # (k-3)^2 class index
ROW_PAIRS = [(i, j) for i in (4, 5, 6) for j in range(KS)]   # oy > 0
COL_PAIRS = [(3, j) for j in (4, 5, 6)]                      # oy = 0, ox > 0

_CACHED = {}
TAP_SET = None   # optional [(i,j)] subset for debugging (unpaired path)


def build_nc(macros=None):
    nc = Bacc()
    x = nc.dram_tensor("x", [C_ALL, HIN, WIN], FP32, kind="ExternalInput")
    ts = nc.dram_tensor("ts", [NHB, NW, 3 * len(ROW_PAIRS), CF * WD], F16,
                        kind="ExternalInput")
    y = nc.dram_tensor("y", [CO, HC, W], FP32, kind="ExternalOutput")

    if macros is None:
        macros = [(hb, wck) for hb in range(NHB) for wck in range(NW)]
    with TileContext(nc) as tc:
        with (
            tc.tile_pool(name="gpool", bufs=1) as gpool,
            tc.tile_pool(name="fpool", bufs=1) as fpool,
            tc.tile_pool(name="stpool", bufs=2) as stpool,
            tc.tile_pool(name="cpool", bufs=1) as cpool,
            tc.tile_pool(name="dpool", bufs=2) as dpool,
            tc.tile_pool(name="spool", bufs=3) as spool,
            tc.psum_pool(name="papool", bufs=1) as papool,
            tc.psum_pool(name="plpool", bufs=4) as plpool,
        ):
            ident = gpool.tile([128, 128], F16, tag="ident", name="ident")
            make_identity(nc, ident[:])
            identN = gpool.tile([128, 128], F16, tag="identN", name="identN")
            nc.vector.tensor_scalar_mul(identN[:], ident[:], -1.0)
            for hb, wcki in macros:
                _macro(nc, tc, x, ts, y, ident, identN, fpool, stpool, cpool,
                       dpool, spool, papool, plpool, hb, wcki)
    nc.compile()
    return nc


def _macro(nc, tc, x, ts, y, ident, identN, fpool, stpool, cpool, dpool,
           spool, papool, plpool, hb, wck):
    w0 = wck * WC
    r0 = hb * 128

    # ---- F tiles: convert the 134-row slab once, DMA-shift 7 views ----
    FA16 = fpool.tile([128, CF * WT], F16, tag="FA16", bufs=1,
                      name=f"FA16_{hb}_{wck}")
    FA3 = FA16[:].rearrange("p (c x) -> p c x", x=WT)
    for half in range(2):
        st = stpool.tile([128, 4 * WT], FP32, tag="stage", bufs=2,
                         name=f"stA_{hb}_{wck}_{half}")
        st3 = st[:].rearrange("p (c x) -> p c x", x=WT)
        eng = nc.sync if half == 0 else nc.scalar
        for ci in range(4):
            c = half * 4 + ci
            eng.dma_start(
                out=st3[:, ci, :],
                in_=x[c, r0 : r0 + 128, w0 : w0 + WT])
        nc.scalar.copy(FA3[:, half * 4 : half * 4 + 4, :], st3[:])
    FB16 = fpool.tile([2 * RAD, CF * WT], F16, tag="FB16", bufs=1,
                      name=f"FB16_{hb}_{wck}")
    FB3 = FB16[:].rearrange("p (c x) -> p c x", x=WT)
    for half in range(2):
        st = stpool.tile([128, 4 * WT], FP32, tag="stage", bufs=2,
                         name=f"stB_{hb}_{wck}_{half}")
        st3 = st[:].rearrange("p (c x) -> p c x", x=WT)
        for ci in range(4):
            c = half * 4 + ci
            nc.sync.dma_start(
                out=st3[0 : 2 * RAD, ci, :],
                in_=x[c, r0 + 128 : r0 + 128 + 2 * RAD, w0 : w0 + WT])
        nc.scalar.copy(FB3[:, half * 4 : half * 4 + 4, :],
                       st3[0 : 2 * RAD, :, :])

    F = [None] * KS  # F[oy]: [128, CF, WT] f16, rows r0+oy .. r0+oy+127
    NCH = {0: CF, 1: CO, 2: CO, 3: CF, 4: CF, 5: CF, 6: CF}
    F[0] = FA16
    # build order follows first tap use; alternate the two HWDGE queues
    for k, oy in enumerate((RAD, 4, 2, 5, 1, 6)):
        ch = NCH[oy]
        Fi = fpool.tile([128, ch * WT], F16, tag=f"F{oy}", bufs=1,
                        name=f"F{oy}_{hb}_{wck}")
        eng = nc.sync if k % 2 == 0 else nc.scalar
        eng.dma_start(out=Fi[0 : 128 - oy, :],
                      in_=FA16[oy : 128, 0 : ch * WT])
        eng.dma_start(out=Fi[128 - oy : 128, :],
                      in_=FB16[0 : oy, 0 : ch * WT])
        F[oy] = Fi

    def f3d(oy):
        return F[oy][:].rearrange("p (c x) -> p c x", x=WT)

    FcW = f3d(RAD)[:, :, RAD : RAD + WD]      # wide center view [128, 8, WD]

    # ---- params (all positive; sign lives in the -I stationary) ----
    # R+ = p^2, sx2 = sx_raw^2, sy2 = sy_raw^2 — via ACT Square from fp32
    R = cpool.tile([128, CF * WC], F16, tag="R", name=f"R_{hb}_{wck}")
    R3 = R[:].rearrange("p (c x) -> p c x", x=WC)
    for half in range(2):
        st = stpool.tile([128, 4 * WT], FP32, tag="stage", bufs=2,
                         name=f"pst_{hb}_{wck}_{half}")
        st3 = st[:].rearrange("p (c x) -> p c x", x=WT)
        for ci in range(4):
            c = half * 4 + ci
            nc.sync.dma_start(
                out=st3[:, ci, 0:WC],
                in_=x[CF + c, r0 + RAD : r0 + RAD + 128,
                      w0 + LPAD : w0 + LPAD + WC])
        nc.scalar.activation(R3[:, half * 4 : half * 4 + 4, :],
                             st3[:, :, 0:WC],
                             mybir.ActivationFunctionType.Square)
    sstt = stpool.tile([128, 4 * WT], FP32, tag="stage", bufs=2,
                       name=f"sst_{hb}_{wck}")
    sst = sstt[:, 0 : 2 * WC]
    for k in range(2):
        nc.sync.dma_start(
            out=sst[:, k * WC : (k + 1) * WC],
            in_=x[2 * CF + k, r0 + RAD : r0 + RAD + 128,
                  w0 + LPAD : w0 + LPAD + WC])
    sxy = cpool.tile([128, 2 * WC], F16, tag="sxy", name=f"sxy_{hb}_{wck}")
    nc.scalar.activation(sxy[:], sst, mybir.ActivationFunctionType.Square)
    sx2 = sxy[:, 0:WC]
    sy2 = sxy[:, WC : 2 * WC]

    # Asp+[(ai,bi)] = A*sx2 + B*sy2, A,B in {0,1,4,9}; (0,0) never used.
    ax = {1: sx2, 2: None, 3: None}
    by = {1: sy2, 2: None, 3: None}
    for k, val in ((2, 4.0), (3, 9.0)):
        t = cpool.tile([128, WC], F16, tag=f"ax{k}", name=f"ax{k}_{hb}_{wck}")
        nc.vector.tensor_scalar_mul(t[:], sx2, val)
        ax[k] = t[:]
        t = cpool.tile([128, WC], F16, tag=f"by{k}", name=f"by{k}_{hb}_{wck}")
        nc.vector.tensor_scalar_mul(t[:], sy2, val)
        by[k] = t[:]
    asp = {}
    for ai in range(4):
        for bi in range(4):
            if ai == 0 and bi == 0:
                continue
            if ai == 0:
                asp[(ai, bi)] = by[bi]
            elif bi == 0:
                asp[(ai, bi)] = ax[ai]
            else:
                t = cpool.tile([128, WC], F16, tag=f"asp{ai}{bi}",
                               name=f"asp{ai}{bi}_{hb}_{wck}")
                nc.vector.tensor_add(t[:], ax[ai], by[bi])
                asp[(ai, bi)] = t[:]

    # ---- persistent PSUM accumulators: wsum + 3 out channels ----
    ones = cpool.tile([128, WC], F16, tag="ones", name=f"ones_{hb}_{wck}")
    nc.vector.memset(ones[:], 1.0)
    accW = papool.tile([128, WC], FP32, tag="accW", name=f"accW_{hb}_{wck}")
    accC = papool.tile([128, CO * WC], FP32, tag="accC",
                       name=f"accC_{hb}_{wck}")
    Fc = f3d(RAD)[:, :, LPAD : LPAD + WC]
    nc.tensor.matmul(out=accW[:], lhsT=ident[:], rhs=ones[:],
                     start=True, stop=False)
    for c in range(CO):
        nc.tensor.matmul(out=accC[:, c * WC : (c + 1) * WC], lhsT=ident[:],
                         rhs=Fc[:, c, :], start=True, stop=False)

    state = {"emitted": 0, "acc": None}
    total = 48 if TAP_SET is None else len(TAP_SET)

    def flush_acc(last):
        w_t, t3 = state["acc"]
        nc.tensor.matmul(out=accW[:], lhsT=ident[:], rhs=w_t[:],
                         start=False, stop=last)
        for c in range(CO):
            nc.tensor.matmul(out=accC[:, c * WC : (c + 1) * WC],
                             lhsT=ident[:],
                             rhs=t3[:, c * WC : (c + 1) * WC],
                             start=False, stop=last)
        state["acc"] = None

    def tap_tail(i, j, rd, uid):
        """logw matmuls -> exp -> t3; acc matmuls deferred one tap so the
        PE stream isn't blocked on this tap's t3 before the next tap's
        logw matmuls."""
        state["emitted"] += 1
        last = state["emitted"] == total
        rd3 = rd[:].rearrange("p (c x) -> p c x", x=WC)
        logw = plpool.tile([128, WC], FP32, tag="logw", bufs=4,
                           name=f"logw_{uid}")
        for c in range(CF):
            nc.tensor.matmul(out=logw[:], lhsT=identN[:], rhs=rd3[:, c, :],
                             start=(c == 0), stop=False)
        nc.tensor.matmul(out=logw[:], lhsT=identN[:],
                         rhs=asp[(IDX4[j], IDX4[i])], start=False, stop=True)
        w_t = spool.tile([128, WC], F16, tag="w", bufs=3, name=f"w_{uid}")
        nc.scalar.activation(w_t[:], logw[:],
                             mybir.ActivationFunctionType.Exp)
        t3 = spool.tile([128, CO * WC], F16, tag="t3", bufs=3,
                        name=f"t3_{uid}")
        w_b = w_t[:].unsqueeze(1).broadcast_to([128, CO, WC])
        nc.vector.tensor_mul(
            t3[:].rearrange("p (c x) -> p c x", x=WC), w_b,
            f3d(i)[:, 0:CO, j + RAD : j + RAD + WC])
        if state["acc"] is not None:
            flush_acc(False)
        state["acc"] = (w_t, t3)
        if last:
            flush_acc(True)

    def rd_mul(d2view, uid):
        rd = dpool.tile([128, CF * WC], F16, tag="rd", bufs=3, name=f"rd_{uid}")
        nc.vector.tensor_mul(
            rd[:].rearrange("p (c x) -> p c x", x=WC), R3, d2view)
        return rd

    def wide_d2(i, j, uid):
        """d2 for tap (i,j) on the wide (518) window, partition = center."""
        d = dpool.tile([128, CF * WD], F16, tag="d", bufs=2, name=f"d_{uid}")
        d3 = d[:].rearrange("p (c x) -> p c x", x=WD)
        nc.vector.tensor_sub(d3, f3d(i)[:, :, j : j + WD], FcW)
        nc.scalar.activation(d[:], d[:], mybir.ActivationFunctionType.Square)
        return d

    if TAP_SET is not None:
        for i, j in TAP_SET:
            uid = f"{hb}_{wck}_{i}_{j}"
            d2w = wide_d2(i, j, uid)
            d23 = d2w[:].rearrange("p (c x) -> p c x", x=WD)
            tap_tail(i, j, rd_mul(d23[:, :, RAD : RAD + WC], uid), uid)
    else:
        pending = None   # deferred partner: (ip, jp, dp3, ox, uidp)
        for s, (i, j) in enumerate(ROW_PAIRS + COL_PAIRS):
            oy, ox = i - RAD, j - RAD
            ip, jp = KS - 1 - i, KS - 1 - j
            uid = f"{hb}_{wck}_{i}_{j}"
            uidp = f"{hb}_{wck}_{ip}_{jp}"
            d2w = wide_d2(i, j, uid)
            d23 = d2w[:].rearrange("p (c x) -> p c x", x=WD)
            # partner d2: d2_{-o}(p) = d2_o(p - o) — row shift via DMA copy
            # (issued immediately so it overlaps the computed tap), col
            # shift via view offset. Strip rows come straight from DRAM.
            if oy == 0:
                dp3 = d23
            else:
                d2s = dpool.tile([128, CF * WD], F16, tag="ds", bufs=6,
                                 name=f"ds_{uidp}")
                eng = nc.scalar if s % 2 == 0 else nc.sync
                eng.dma_start(
                    out=d2s[0 : oy, :],
                    in_=ts[hb, wck, 3 * s + RAD - oy : 3 * s + RAD, :])
                eng.dma_start(out=d2s[oy : 128, :],
                              in_=d2w[0 : 128 - oy, :])
                dp3 = d2s[:].rearrange("p (c x) -> p c x", x=WD)
            # run the PREVIOUS pair's partner first: its DMA has had a full
            # pair of work to complete, and its rd'/t3 land early in the
            # Vector stream (they free the ds/rd rings the DMAs wait on)
            if pending is not None:
                pi, pj, pdp3, pox, puid = pending
                tap_tail(pi, pj,
                         rd_mul(pdp3[:, :, RAD - pox : RAD - pox + WC],
                                puid), puid)
            # computed member
            tap_tail(i, j, rd_mul(d23[:, :, RAD : RAD + WC], uid), uid)
            pending = (ip, jp, dp3, ox, uidp)
        pi, pj, pdp3, pox, puid = pending
        tap_tail(pi, pj,
                 rd_mul(pdp3[:, :, RAD - pox : RAD - pox + WC], puid), puid)

    # ---- out = acc / wsum ----
    rec = spool.tile([128, WC], FP32, tag="rec", bufs=1,
                     name=f"rec_{hb}_{wck}")
    nc.vector.reciprocal_approx_fast(rec[:], accW[:])
    out3 = spool.tile([128, CO * WC], FP32, tag="out3", bufs=1,
                      name=f"out3_{hb}_{wck}")
    for c in range(CO):
        nc.vector.tensor_mul(out3[:, c * WC : (c + 1) * WC], rec[:],
                             accC[:, c * WC : (c + 1) * WC])
    o3 = out3[:].rearrange("p (c x) -> p c x", x=WC)
    for c in range(CO):
        nc.sync.dma_start(out=y[c, r0 : r0 + 128, w0 : w0 + WC],
                          in_=o3[:, c, :])


def shard_inputs(input):
    """input [2,18,1024,1024] -> 8 slabs [18, 262, 1036] + d2 strips."""
    input = np.asarray(input, dtype=np.float32)
    per_b = 4
    rows = H // per_b
    in_maps = []
    for core in range(8):
        b, q = divmod(core, per_b)
        r0 = q * rows
        slab = np.full((C_ALL, HIN, WIN), SENT, dtype=np.float32)
        s_lo = max(r0 - RAD, 0)
        s_hi = min(r0 + rows + RAD, H)
        slab[:, s_lo - (r0 - RAD) : s_hi - (r0 - RAD), LPAD : LPAD + W] = \
            input[b, :, s_lo:s_hi, :]
        # strips: d2 of the 3 rows above each 128-row block, per row-pair tap
        ts = np.empty((NHB, NW, 3 * len(ROW_PAIRS), CF * WD), np.float16)
        f = slab[:CF]
        for hb in range(NHB):
            for wck in range(NW):
                w0 = wck * WC
                for s, (i, j) in enumerate(ROW_PAIRS):
                    oy, ox = i - RAD, j - RAD
                    # strip row m (0..2) = center slab row hb*128 + m,
                    # col v (0..WD-1) = slab col w0 + 3 + v
                    q0 = hb * 128
                    c0 = w0 + RAD
                    d = (f[:, q0 + oy : q0 + oy + RAD,
                           c0 + ox : c0 + ox + WD]
                         - f[:, q0 : q0 + RAD, c0 : c0 + WD])
                    ts[hb, wck, 3 * s : 3 * s + 3] = \
                        (d.astype(np.float32) ** 2).transpose(1, 0, 2) \
                        .reshape(RAD, CF * WD).astype(np.float16)
        in_maps.append({"x": np.ascontiguousarray(slab), "ts": ts})
    return in_maps


def assemble(results):
    out = np.empty((B, CO, H, W), dtype=np.float32)
    rows = H // 4
    for core in range(8):
        b, q = divmod(core, 4)
        out[b, :, q * rows : (q + 1) * rows, :] = results[core]["y"]
    return out


def kernel(input):
    from concourse.bass_utils import run_bass_kernel_spmd

    if "nc" not in _CACHED:
        _CACHED["nc"] = build_nc()
    in_maps = shard_inputs(input)
    res = run_bass_kernel_spmd(_CACHED["nc"], in_maps, list(range(8)))
    return assemble(res.results)


# revision 10
# speedup vs baseline: 1.9009x; 1.9009x over previous
"""Bilateral filter (7x7, dilation 1) Trainium2 Bass kernel — v8.

Problem: input [2, 18, 1024, 1024] f32.
  filterable = input[:, :8]; params = -(input[:, 8:]**2)
  range coeffs = params[:, :8], sx = params[:, 8], sy = params[:, 9]
  out[c] = sum_taps w * f_c(shifted) / sum_taps w, c < 3
  w = exp(sum_c r_c (fn_c - f_c)^2 + sx dx^2 + sy dy^2), OOB taps masked.

Sharding: data-parallel over (batch, H): 8 cores, each 256 rows of one batch
image (+3 halo rows / cols, sentinel-padded host-side; 4-col left pad).

v8 design (v7 measured 1241us: DVE 1113us busy, ACT 958, PE 728):
  * per tap: DVE sub -> ACT Square -> DVE rd+ = R+ * d2 -> PE: 8 neg-identity
    matmuls + 1 Asp+ matmul accumulate -logw in a PSUM bank -> ACT Exp reads
    PSUM -> DVE t3 = w * [f0,f1,f2] -> PE +identity matmuls accumulate w and
    t3 into persistent PSUM acc banks across all 49 taps.
  * all-positive moving data (R+ = p^2 via one ACT Square, Asp+ = a*sx2+b*sy2
    via 4x tensor_scalar + 2x tensor_add), sign flip lives in the -I
    stationary: kills the 1x-mode scalar_tensor_tensor prep of v6/v7.
  * F tiles: one fp32->f16 conversion per 134-row slab (ACT, 2 big copies),
    then the 7 row-shifted working tiles are partition-offset SBUF->SBUF
    DMA copies (FA16 + 6-row FB16 tail); F[0] aliases FA16. Replaces 56
    per-macro ACT conversions and 4x the HBM traffic.
  * sentinel 24.0: r*d^2 finite in fp16 (no -inf: 0 * -inf = NaN in the PE).
  * PSUM: accW 1 + accC 3 + logw rotation 4 = 8 banks.
"""

import sys

if "/opt/trn_rl_repo" not in sys.path:
    sys.path.insert(0, "/opt/trn_rl_repo")

import numpy as np

import concourse.bass as bass
import concourse.mybir as mybir
from concourse.bacc import Bacc
from concourse.masks import make_identity
from concourse.tile import TileContext

FP32 = mybir.dt.float32
F16 = mybir.dt.float16

B, C_ALL, H, W = 2, 18, 1024, 1024
CF = 8                      # filterable channels
CO = 3                      # output channels
KS, RAD = 7, 3
HC = H * B // 8             # 256 output rows per core
HIN = HC + 2 * RAD          # 262 input rows per core (halo padded host-side)
LPAD = 4                    # left col pad (4B alignment for f16 2x mode)
WIN = W + LPAD + RAD + 1    # 1032 input cols per core (halo padded host-side)
WC = 512                    # W chunk (= one PSUM bank of fp32)
NW = W // WC                # 2
NHB = HC // 128             # 2
WT = WC + LPAD + RAD + 1    # 520 = chunk + col halo (even)
SENT = 24.0                 # sentinel: exp(-p2*d^2 sum) == 0, finite in fp16
IDX4 = [3, 2, 1, 0, 1, 2, 3]                          # (k-3)^2 class index

_CACHED = {}
TAP_SET = None   # optional [(i,j)] subset for debugging


def build_nc(macros=None):
    nc = Bacc()
    x = nc.dram_tensor("x", [C_ALL, HIN, WIN], FP32, kind="ExternalInput")
    y = nc.dram_tensor("y", [CO, HC, W], FP32, kind="ExternalOutput")

    if macros is None:
        macros = [(hb, wck) for hb in range(NHB) for wck in range(NW)]
    with TileContext(nc) as tc:
        with (
            tc.tile_pool(name="gpool", bufs=1) as gpool,
            tc.tile_pool(name="fpool", bufs=1) as fpool,
            tc.tile_pool(name="stpool", bufs=2) as stpool,
            tc.tile_pool(name="cpool", bufs=1) as cpool,
            tc.tile_pool(name="dpool", bufs=2) as dpool,
            tc.tile_pool(name="spool", bufs=3) as spool,
            tc.psum_pool(name="papool", bufs=1) as papool,
            tc.psum_pool(name="plpool", bufs=4) as plpool,
        ):
            ident = gpool.tile([128, 128], F16, tag="ident", name="ident")
            make_identity(nc, ident[:])
            identN = gpool.tile([128, 128], F16, tag="identN", name="identN")
            nc.vector.tensor_scalar_mul(identN[:], ident[:], -1.0)
            for hb, wcki in macros:
                _macro(nc, tc, x, y, ident, identN, fpool, stpool, cpool,
                       dpool, spool, papool, plpool, hb, wcki)
    nc.compile()
    return nc


def _macro(nc, tc, x, y, ident, identN, fpool, stpool, cpool, dpool, spool,
           papool, plpool, hb, wck):
    w0 = wck * WC
    r0 = hb * 128

    # ---- param DMAs first: small, lets R/Asp prep overlap F loads ----
    pst = stpool.tile([128, CF * WC], FP32, tag="pstage", bufs=1,
                      name=f"pst_{hb}_{wck}")
    pst3 = pst[:].rearrange("p (c x) -> p c x", x=WC)
    for c in range(CF):
        nc.sync.dma_start(
            out=pst3[:, c, :],
            in_=x[CF + c, r0 + RAD : r0 + RAD + 128, w0 + LPAD : w0 + LPAD + WC])
    sst = stpool.tile([128, 2 * WC], FP32, tag="sstage", bufs=1,
                      name=f"sst_{hb}_{wck}")
    for k in range(2):
        nc.sync.dma_start(
            out=sst[:, k * WC : (k + 1) * WC],
            in_=x[2 * CF + k, r0 + RAD : r0 + RAD + 128,
                  w0 + LPAD : w0 + LPAD + WC])

    # ---- F tiles: convert the 134-row slab once, DMA-shift 7 views ----
    FA16 = fpool.tile([128, CF * WT], F16, tag="FA16", bufs=1,
                      name=f"FA16_{hb}_{wck}")
    FA3 = FA16[:].rearrange("p (c x) -> p c x", x=WT)
    for half in range(2):
        st = stpool.tile([128, 4 * WT], FP32, tag="stage", bufs=2,
                         name=f"stA_{hb}_{wck}_{half}")
        st3 = st[:].rearrange("p (c x) -> p c x", x=WT)
        for ci in range(4):
            c = half * 4 + ci
            nc.sync.dma_start(
                out=st3[:, ci, :],
                in_=x[c, r0 : r0 + 128, w0 : w0 + WT])
        nc.scalar.copy(FA3[:, half * 4 : half * 4 + 4, :], st3[:])
    FB16 = fpool.tile([2 * RAD, CF * WT], F16, tag="FB16", bufs=1,
                      name=f"FB16_{hb}_{wck}")
    FB3 = FB16[:].rearrange("p (c x) -> p c x", x=WT)
    stB = stpool.tile([2 * RAD, CF * WT], FP32, tag="stageB", bufs=1,
                      name=f"stB_{hb}_{wck}")
    stB3 = stB[:].rearrange("p (c x) -> p c x", x=WT)
    for c in range(CF):
        nc.sync.dma_start(
            out=stB3[:, c, :],
            in_=x[c, r0 + 128 : r0 + 128 + 2 * RAD, w0 : w0 + WT])
    nc.scalar.copy(FB16[:], stB[:])

    F = [None] * KS  # F[oy]: [128, CF, WT] f16, rows r0+oy .. r0+oy+127
    F[0] = FA16
    for oy in (RAD, 1, 2, 4, 5, 6):
        Fi = fpool.tile([128, CF * WT], F16, tag=f"F{oy}", bufs=1,
                        name=f"F{oy}_{hb}_{wck}")
        nc.sync.dma_start(out=Fi[0 : 128 - oy, :], in_=FA16[oy : 128, :])
        nc.sync.dma_start(out=Fi[128 - oy : 128, :], in_=FB16[0 : oy, :])
        F[oy] = Fi

    def f3d(oy):
        return F[oy][:].rearrange("p (c x) -> p c x", x=WT)

    Fc = f3d(RAD)[:, :, LPAD : LPAD + WC]     # center view [128, 8, WC]

    # ---- params (all positive; sign lives in the -I stationary) ----
    # R+ = p^2 (one ACT Square), sx2 = sx_raw^2, sy2 = sy_raw^2
    R = cpool.tile([128, CF * WC], F16, tag="R", name=f"R_{hb}_{wck}")
    nc.scalar.activation(R[:], pst[:], mybir.ActivationFunctionType.Square)
    sxy = cpool.tile([128, 2 * WC], F16, tag="sxy", name=f"sxy_{hb}_{wck}")
    nc.scalar.activation(sxy[:], sst[:], mybir.ActivationFunctionType.Square)
    sx2 = sxy[:, 0:WC]
    sy2 = sxy[:, WC : 2 * WC]

    # Asp+[(ai,bi)] = A*sx2 + B*sy2, A,B in {0,1,4,9}; (0,0) never used.
    ax = {1: sx2, 2: None, 3: None}
    by = {1: sy2, 2: None, 3: None}
    for k, val in ((2, 4.0), (3, 9.0)):
        t = cpool.tile([128, WC], F16, tag=f"ax{k}", name=f"ax{k}_{hb}_{wck}")
        nc.vector.tensor_scalar_mul(t[:], sx2, val)
        ax[k] = t[:]
        t = cpool.tile([128, WC], F16, tag=f"by{k}", name=f"by{k}_{hb}_{wck}")
        nc.vector.tensor_scalar_mul(t[:], sy2, val)
        by[k] = t[:]
    asp = {}
    for ai in range(4):
        for bi in range(4):
            if ai == 0 and bi == 0:
                continue
            if ai == 0:
                asp[(ai, bi)] = by[bi]
            elif bi == 0:
                asp[(ai, bi)] = ax[ai]
            else:
                t = cpool.tile([128, WC], F16, tag=f"asp{ai}{bi}",
                               name=f"asp{ai}{bi}_{hb}_{wck}")
                nc.vector.tensor_add(t[:], ax[ai], by[bi])
                asp[(ai, bi)] = t[:]

    # ---- persistent PSUM accumulators: wsum + 3 out channels ----
    # center tap (w=1) folded into the start=True init matmuls.
    ones = cpool.tile([128, WC], F16, tag="ones", name=f"ones_{hb}_{wck}")
    nc.vector.memset(ones[:], 1.0)
    accW = papool.tile([128, WC], FP32, tag="accW", name=f"accW_{hb}_{wck}")
    accC = papool.tile([128, CO * WC], FP32, tag="accC",
                       name=f"accC_{hb}_{wck}")
    nc.tensor.matmul(out=accW[:], lhsT=ident[:], rhs=ones[:],
                     start=True, stop=False)
    for c in range(CO):
        nc.tensor.matmul(out=accC[:, c * WC : (c + 1) * WC], lhsT=ident[:],
                         rhs=Fc[:, c, :], start=True, stop=False)

    # ---- 48 off-center taps ----
    taps = TAP_SET if TAP_SET is not None else [
        (i, j) for i in range(KS) for j in range(KS) if (i, j) != (RAD, RAD)]
    n_taps = len(taps)
    for ti, (i, j) in enumerate(taps):   # oy = i - 3, ox = j - 3
        last = ti == n_taps - 1
        sh = f3d(i)[:, :, j + 1 : j + 1 + WC]  # shifted view [128, 8, WC]
        d = dpool.tile([128, CF * WC], F16, tag="d",
                       name=f"d_{hb}_{wck}_{i}_{j}")
        nc.vector.tensor_sub(
            d[:].rearrange("p (c x) -> p c x", x=WC), sh, Fc)
        d2 = d
        nc.scalar.activation(d2[:], d[:],
                             mybir.ActivationFunctionType.Square)
        rd = dpool.tile([128, CF * WC], F16, tag="rd",
                        name=f"rd_{hb}_{wck}_{i}_{j}")
        nc.vector.tensor_mul(rd[:], R[:], d2[:])
        rd3 = rd[:].rearrange("p (c x) -> p c x", x=WC)
        # PE: -(channel reduce + Asp) accumulated in a PSUM bank
        logw = plpool.tile([128, WC], FP32, tag="logw", bufs=4,
                           name=f"logw_{hb}_{wck}_{i}_{j}")
        for c in range(CF):
            nc.tensor.matmul(out=logw[:], lhsT=identN[:], rhs=rd3[:, c, :],
                             start=(c == 0), stop=False)
        nc.tensor.matmul(out=logw[:], lhsT=identN[:],
                         rhs=asp[(IDX4[j], IDX4[i])], start=False, stop=True)
        w_t = spool.tile([128, WC], F16, tag="w",
                         name=f"w_{hb}_{wck}_{i}_{j}")
        nc.scalar.activation(w_t[:], logw[:],
                             mybir.ActivationFunctionType.Exp)
        # numerator: t3 = w * [f0, f1, f2]
        t3 = spool.tile([128, CO * WC], F16, tag="t3", bufs=3,
                        name=f"t3_{hb}_{wck}_{i}_{j}")
        w_b = w_t[:].unsqueeze(1).broadcast_to([128, CO, WC])
        nc.vector.tensor_mul(
            t3[:].rearrange("p (c x) -> p c x", x=WC), w_b,
            f3d(i)[:, 0:CO, j + 1 : j + 1 + WC])
        nc.tensor.matmul(out=accW[:], lhsT=ident[:], rhs=w_t[:],
                         start=False, stop=last)
        for c in range(CO):
            nc.tensor.matmul(out=accC[:, c * WC : (c + 1) * WC], lhsT=ident[:],
                             rhs=t3[:, c * WC : (c + 1) * WC],
                             start=False, stop=last)

    # ---- out = acc / wsum ----
    rec = spool.tile([128, WC], FP32, tag="rec", bufs=1,
                     name=f"rec_{hb}_{wck}")
    nc.vector.reciprocal_approx_fast(rec[:], accW[:])
    out3 = spool.tile([128, CO * WC], FP32, tag="out3", bufs=1,
                      name=f"out3_{hb}_{wck}")
    for c in range(CO):
        nc.vector.tensor_mul(out3[:, c * WC : (c + 1) * WC], rec[:],
                             accC[:, c * WC : (c + 1) * WC])
    o3 = out3[:].rearrange("p (c x) -> p c x", x=WC)
    for c in range(CO):
        nc.sync.dma_start(out=y[c, r0 : r0 + 128, w0 : w0 + WC],
                          in_=o3[:, c, :])


def shard_inputs(input):
    """input [2,18,1024,1024] -> 8 per-core slabs [18, 262, 1030]."""
    input = np.asarray(input, dtype=np.float32)
    per_b = 4
    rows = H // per_b
    in_maps = []
    for core in range(8):
        b, q = divmod(core, per_b)
        r0 = q * rows
        slab = np.full((C_ALL, HIN, WIN), SENT, dtype=np.float32)
        s_lo = max(r0 - RAD, 0)
        s_hi = min(r0 + rows + RAD, H)
        slab[:, s_lo - (r0 - RAD) : s_hi - (r0 - RAD), LPAD : LPAD + W] = \
            input[b, :, s_lo:s_hi, :]
        in_maps.append({"x": np.ascontiguousarray(slab)})
    return in_maps


def assemble(results):
    out = np.empty((B, CO, H, W), dtype=np.float32)
    rows = H // 4
    for core in range(8):
        b, q = divmod(core, 4)
        out[b, :, q * rows : (q + 1) * rows, :] = results[core]["y"]
    return out


def kernel(input):
    from concourse.bass_utils import run_bass_kernel_spmd

    if "nc" not in _CACHED:
        _CACHED["nc"] = build_nc()
    in_maps = shard_inputs(input)
    res = run_bass_kernel_spmd(_CACHED["nc"], in_maps, list(range(8)))
    return assemble(res.results)
